# revision 84
# baseline (speedup 1.0000x reference)
"""Multi-head attention (B=2, S=2048, D=1024, H=16) on 8 Trainium2 cores.

Sharding: core c handles batch b = c//4 and head group g = c%4 (4 heads).
Output projection is row-sharded over head dims; per-core partial outputs
are summed on the host (bias added on the host).

Design (PE-paced, exp split across engines): the softmax exp (128
[128,1024] tiles/core) is the single largest engine load; it is split
between ACT (exact exp, h0 + two h1 steps) and DVE (Schraudolph bit-trick
fast-exp: bf16 bits = int16(round(x*128/ln2 + (127*128-7.3))), one
tensor_scalar per tile, ~+-3%% ripple that averages out over j; adds
~6e-3 rel err against a 2e-2 tolerance).  With both exp engines just
below the PE's pace, the PE becomes the pacer and must never idle: an
idle gap re-throttles the HAM clock gate (k=4 for ~3.4us = 2x slower
matmuls).  Thin steps get garbage fills.

  - psS: THREE [128,1024] f32 slots (6 PSUM banks); two QK tiles per
    step, 3-slot rotation opens each QK's WAR gate ~1.5 steps early.
  - PV: prev-block h1 4 jb/step at steps 3-6; own h0 2 jb/step at steps
    9-15 (jb15 + recips deferred to the next block's step 0).  h1's exp
    runs on ACT only at steps 1 (the DVE runs norm muls there, so the
    scheduler cannot hoist them ahead of an exp), 6 and 8.
  - norms in phases so the psO-ring handoff is off the critical path:
    recip halves (DVE) right after the PV chain, gpsimd broadcast halves,
    DVE mul halves one step later (behind that step's exp in the in-order
    DVE queue).  h1 writes at_sb rows 64-127 in place; h0 stages a nm
    tile and row-shifts via one SBUF DMA.
  - step 15 of block 3 is custom: h0's exp goes through the fast-exp so
    PV(3,0) closes and norm(3,0)'s recips run in-step; h1's exp runs on
    the (idle) ACT; PV(3,1)'s first 512-col sweep accumulates into a
    freed psS slot in the same step.
  - tail: PV(3,1)'s second sweep lands in the psO tile (free once
    norm(3,0) reads retire) so the psS slot frees right after the first
    norm-mul and the 16-unit projection keeps a depth-3 rotation.  Each
    unit: 4 matmuls, one full-width [128,1024] psum->sbuf bf16 copy
    (ACT/DVE alternating), one y DMA.  Norm halves lead the DVE queue so
    hi units' at_sb gates clear while earlier units are still on the PE.
    The kernel ends with ~10us of y write drain (~190GB/s effective).
"""

import sys

sys.path.insert(0, "/opt/trn_rl_repo")

from contextlib import ExitStack

import numpy as np
import ml_dtypes

import concourse.bass as bass
import concourse.tile as tile
from concourse import bacc, mybir
from concourse.alu_op_type import AluOpType

N_CORES = 8
B, S, D_MODEL = 2, 2048, 1024
NUM_HEADS, D_K = 16, 64
H_PER_CORE = 4
SCALE = D_K ** -0.5
IS = 1024                 # i-super width
JB = S // 128             # 16 j-blocks
VA_W = 128                # ones col 0, zeros 1-63, v at 64-127
VA_CHUNK = 4              # va split into 4 tiles of 4 j-blocks each
ET_BUFS = 52

F32 = mybir.dt.float32
BF16 = mybir.dt.bfloat16
I16 = mybir.dt.int16
AF = mybir.ActivationFunctionType
BLOCKS = [(0, 0), (1, 0), (0, 1024), (1, 1024)]  # (pair, i0), iw = 1024

# DVE fast-exp (Schraudolph bit trick in bf16): bf16 bits of exp(x) ~=
# int16(round(x * 128/ln2 + (127*128 - C))).  HW f32->i16 convert rounds
# to nearest (verified).  C tuned offline so the mean approximation error
# is ~0 (softmax mixes fast-exp and exact-exp j-blocks, so a nonzero mean
# biases block weights; the residual +-3% ripple averages out over j).
S16F = 128.0 / float(np.log(2.0))
EXP_BIAS = 127.0 * 128.0 - 7.3

PE_TARGET = 1650          # ns of PE time to emit per step (blocks 1-3)
PE_TARGET_B0 = 1300       # block 0 runs DVE/PE-paced, not ACT-paced
QK_PE = 713               # measured QK quad wall (LDW stagger included)
PV_PE = 426               # per j-block (2 chunk matmuls)
FILL_MAC = 213

# blocks 1-3: which steps route h1's exp to the DVE fast-exp.  Steps with
# h1 on ACT serialize QK -> exp_h0 -> exp_h1 -> next QK's psS WAR into a
# ~2.3us loop, so only three isolated steps keep h1 on ACT: 0 and 6 (the
# DVE runs the norm recips there) and 8 (the norm muls).
DVE_H1_STEPS = {0, 2, 3, 4, 5, 7, 9, 10, 11, 12, 13, 14, 15}

# prev-block h1 PV: 4 jb/step at steps 3-6 (step 2 would race the psO
# release by norm(t-1,0)'s muls, which run at step 1 AFTER that step's
# exp in the DVE queue); norm(t-1,1) recips+broadcast at step 6, muls at
# step 8, PV(t,0) from step 9.
PV1_SCHED = {3: [0, 1, 2, 3], 4: [4, 5, 6, 7], 5: [8, 9, 10, 11],
             6: [12, 13, 14, 15]}
# own h0 PV: starts at step 9 (after the psO handoff), jb 15 deferred
PV0_SCHED = {9: [0, 1], 10: [2, 3], 11: [4, 5], 12: [6, 7], 13: [8, 9],
             14: [10, 11], 15: [12, 13, 14]}


def ds(start, size):
    return slice(start, start + size)


def _trace(ctx: ExitStack, tc: tile.TileContext, io: dict):
    nc = tc.nc

    const = ctx.enter_context(tc.tile_pool(name="const", bufs=1))
    etp = ctx.enter_context(tc.tile_pool(name="et", bufs=ET_BUFS))
    normp = ctx.enter_context(tc.tile_pool(name="norm", bufs=2))
    atp = ctx.enter_context(tc.tile_pool(name="at", bufs=1))
    youtp = ctx.enter_context(tc.tile_pool(name="yout", bufs=6))
    miscp = ctx.enter_context(tc.tile_pool(name="misc", bufs=2))
    psS = ctx.enter_context(tc.tile_pool(name="psS", bufs=3, space="PSUM"))
    psO = ctx.enter_context(tc.tile_pool(name="psO", bufs=1, space="PSUM"))

    # ---- resident inputs (order matters: earliest-needed first) ----
    kt_sb = [const.tile([128, S], BF16, tag=f"kt{p}", name=f"kt{p}")
             for p in range(2)]
    qt_sb = [const.tile([128, S], BF16, tag=f"qt{p}", name=f"qt{p}")
             for p in range(2)]
    va_sb = [const.tile([128, JB // VA_CHUNK * H_PER_CORE * VA_W], BF16,
                        tag=f"va{c}", name=f"va{c}") for c in range(VA_CHUNK)]
    # critical-first DMA order: only ~1.5MB is needed in the first ~15us
    # (kt[0], qt[0][:, :1024], va chunk 0); the other ~3MB would otherwise
    # share HBM bandwidth with it.  The bulk issues sit BEHIND the warm-out
    # DMA on the in-order sync queue, and that DMA waits on the warmup
    # matmuls (~13us) - an artificial gate that delays the bulk transfers.
    nc.sync.dma_start(kt_sb[0][:, 0:128], io["kt"][0][:, 0:128])
    nc.sync.dma_start(qt_sb[0][:, 0:512], io["qt"][0][:, 0:512])
    nc.sync.dma_start(qt_sb[0][:, 512:IS], io["qt"][0][:, 512:IS])
    nc.sync.dma_start(kt_sb[0][:, 128:512], io["kt"][0][:, 128:512])
    nc.sync.dma_start(kt_sb[0][:, 512:1024], io["kt"][0][:, 512:1024])
    nc.sync.dma_start(va_sb[0][:], io["va"][0])
    nc.sync.dma_start(kt_sb[0][:, 1024:S], io["kt"][0][:, 1024:S])
    at_sb = [atp.tile([128, S], BF16, tag=f"at{p}", name=f"at{p}")
             for p in range(2)]

    # ---- warmup ----
    # exp-table preload on ACT (reads the first kt sliver, so the ~2.7us
    # table load overlaps input DMA), then a run of garbage matmuls on the
    # first kt sliver only, ramping the PE p-state/HAM while qt streams in
    wexp = miscp.tile([1, 128], F32, tag="warm_exp", name="warm_exp")
    nc.scalar.activation(wexp[:], kt_sb[0][0:1, 0:128], AF.Exp, scale=SCALE)
    nc.sync.dma_start(io["wexp"][:], wexp[:])
    wps = psS.tile([128, IS], F32, tag="S", name="warm_ps")
    for i in range(20):
        nc.tensor.matmul(wps[:, ds(128 * (i % 8), 128)],
                         kt_sb[0][:, 0:128], kt_sb[0][:, 0:128],
                         start=True, stop=True, skip_group_check=True)
    wsb = miscp.tile([1, 128], F32, tag="warm_out", name="warm_out")
    nc.vector.tensor_copy(wsb[:], wps[0:1, 0:128])
    nc.sync.dma_start(io["warm"][:], wsb[:])

    wt_sb = []
    nc.sync.dma_start(va_sb[1][:], io["va"][1])
    nc.sync.dma_start(kt_sb[1][:], io["kt"][1])
    nc.sync.dma_start(qt_sb[1][:, 0:IS], io["qt"][1][:, 0:IS])
    nc.sync.dma_start(va_sb[2][:], io["va"][2])
    nc.sync.dma_start(va_sb[3][:], io["va"][3])
    for p in range(2):
        t = const.tile([128, D_MODEL], BF16, tag=f"wt{p}")
        nc.sync.dma_start(t[:], io["wt"][p])
        wt_sb.append(t)
    nc.sync.dma_start(qt_sb[0][:, IS:S], io["qt"][0][:, IS:S])
    nc.sync.dma_start(qt_sb[1][:, IS:S], io["qt"][1][:, IS:S])

    ET = {}    # (block_idx, h2) -> list of 16 E tiles
    PSO = {}   # (block_idx, h2) -> psum tile
    RR = {}    # (block_idx, h2) -> [rr half tiles]
    dead_sp = [wps]  # S tiles whose exp already ran: fill targets

    def exp_on_dve(t, jb, h2):
        if h2 == 0:
            # block 3's very last h0 tile goes through the fast-exp so its
            # PV + norm(3,0) recips can start within step 15
            return t == 3 and jb == 15
        return t == 0 or jb in DVE_H1_STEPS

    def emit_qk(t, jb):
        pr, i0 = BLOCKS[t]
        # interleave the two heads' matmuls: distinct PE row-groups run
        # concurrently (full-array MAC activity keeps HAM at 8/8)
        sps = [psS.tile([128, IS], F32, tag="S", name="sp") for _ in range(2)]
        for nch in range(2):
            for h2 in range(2):
                nc.tensor.matmul(
                    sps[h2][:, ds(nch * 512, 512)],
                    kt_sb[pr][ds(h2 * 64, 64), ds(jb * 128, 128)],
                    qt_sb[pr][ds(h2 * 64, 64), ds(i0 + nch * 512, 512)],
                    start=True, stop=True,
                )
        return sps

    def emit_exp(t, jb, h2, sps):
        e = etp.tile([128, IS], BF16, tag="et", name="e")
        if exp_on_dve(t, jb, h2):
            nc.vector.tensor_scalar(
                e[:].bitcast(I16), sps[h2][:],
                float(SCALE * S16F), float(EXP_BIAS),
                AluOpType.mult, AluOpType.add)
        else:
            nc.scalar.activation(e[:], sps[h2][:], AF.Exp, scale=SCALE)
        ET[(t, h2)][jb] = e
        dead_sp.append(sps[h2])

    def emit_qk_exp(t, jb):
        sps = emit_qk(t, jb)
        emit_exp(t, jb, 0, sps)
        emit_exp(t, jb, 1, sps)

    def emit_pv(t, h2, jbps, pool=None):
        pr, i0 = BLOCKS[t]
        h = pr * 2 + h2
        if (t, h2) not in PSO:
            pool = pool or psO
            tag = "O" if pool is psO else "S"
            PSO[(t, h2)] = pool.tile([128, IS], F32, tag=tag, name="psO")
        O = PSO[(t, h2)]
        for jbp in jbps:
            va = va_sb[jbp // VA_CHUNK]
            vo = (jbp % VA_CHUNK) * H_PER_CORE * VA_W + h * VA_W
            for nch in range(2):
                nc.tensor.matmul(
                    O[0:128, ds(nch * 512, 512)],
                    va[:, ds(vo, VA_W)],
                    ET[(t, h2)][jbp][:, ds(nch * 512, 512)],
                    start=(jbp == 0), stop=(jbp == JB - 1),
                    skip_group_check=True,
                )

    # ---- norm phases ----
    # recip halves as soon as the PV chain retires, each immediately
    # broadcast to partitions 64-127 by the (otherwise idle) gpsimd; the
    # DVE mul halves run in the NEXT step's pre-QK phase (ahead of that
    # step's exp in the in-order DVE queue) and release the psO slot
    def emit_norm_recip(t, h2):
        O = PSO[(t, h2)]
        rr = normp.tile([1, IS], F32, tag="rr", name="rr")
        bct = normp.tile([128, IS], F32, tag="bc", name="bc")
        for off in (0, 512):
            nc.vector.reciprocal_approx_fast(rr[0:1, ds(off, 512)],
                                             O[0:1, ds(off, 512)])
            nc.gpsimd.partition_broadcast(bct[:, ds(off, 512)],
                                          rr[0:1, ds(off, 512)])
        RR[(t, h2)] = bct

    def emit_norm_mul(t, h2):
        pr, i0 = BLOCKS[t]
        O = PSO[(t, h2)]
        bct = RR[(t, h2)]
        if h2 == 1:
            for off in (0, 512):
                nc.vector.tensor_mul(
                    at_sb[pr][ds(64, 64), ds(i0 + off, 512)],
                    O[64:128, ds(off, 512)], bct[64:128, ds(off, 512)])
        else:
            nm = normp.tile([128, IS], BF16, tag="nm", name="nm")
            for off in (0, 512):
                nc.vector.tensor_mul(nm[ds(64, 64), ds(off, 512)],
                                     O[64:128, ds(off, 512)],
                                     bct[64:128, ds(off, 512)])
            nc.sync.dma_start(at_sb[pr][ds(0, 64), ds(i0, IS)], nm[64:128, :])
        del ET[(t, h2)]
        del RR[(t, h2)]

    def emit_proj(ic, eng="vector", mochs=(0, 1)):
        # one i-chunk through a free psS slot; mochs filters out the
        # 512-wide halves already produced inline.  One full-width copy
        # per unit, engines alternating between units (one [128,1024] op
        # beats two 512 halves on per-op overhead)
        Y = psS.tile([128, IS], F32, tag="S", name="Ypj")
        for moch in mochs:
            for hd2 in range(2):
                nc.tensor.matmul(
                    Y[:, ds(moch * 512, 512)],
                    at_sb[hd2][:, ds(ic * 128, 128)],
                    wt_sb[hd2][:, ds(moch * 512, 512)],
                    start=(hd2 == 0), stop=(hd2 == 1),
                    skip_group_check=True,
                )
        ysb = youtp.tile([128, IS], BF16, tag="y")
        if len(mochs) == 2:
            if eng == "vector":
                nc.vector.tensor_copy(ysb[:], Y[:])
            else:
                nc.scalar.copy(ysb[:], Y[:])
            nc.sync.dma_start(io["y"][ds(ic * 128, 128), :], ysb[:])
        else:
            for moch in mochs:
                sl = ds(moch * 512, 512)
                if eng == "vector":
                    nc.vector.tensor_copy(ysb[:, sl], Y[:, sl])
                else:
                    nc.scalar.copy(ysb[:, sl], Y[:, sl])
                nc.sync.dma_start(io["y"][ds(ic * 128, 128), sl], ysb[:, sl])

    def emit_fill(mac_ns, half=None):
        # full-array garbage matmuls (K=128, 512 cols): keep the PE's MAC
        # duty above the HAM re-throttle threshold.  Target the [-3]
        # dead S-tile: its exp is done and its psS slot is not
        # re-allocated until the next step's h0 QK.
        tgt = dead_sp[-3] if len(dead_sp) >= 3 else dead_sp[0]
        n = max(0, round(mac_ns / FILL_MAC))
        for i in range(n):
            off = 512 * (i % 2 if half is None else half)
            nc.tensor.matmul(tgt[:, ds(off, 512)],
                             kt_sb[0][:, 0:128], kt_sb[0][:, 0:512],
                             start=True, stop=True, skip_group_check=True)

    inline_done = set()

    def emit_pv31_sweep(nch):
        # nch0 accumulates in a psS slot during step 15; nch1 in the psO
        # tile (free once norm(3,0) reads retire), so the psS slot frees
        # right after nch0's norm-mul and the proj rotation keeps depth 3
        key = (3, 1) if nch == 0 else (3, 1, "b")
        if key not in PSO:
            if nch == 0:
                PSO[key] = psS.tile([128, IS], F32, tag="S", name="psO31")
            else:
                PSO[key] = psO.tile([128, IS], F32, tag="O", name="psO31b")
        O = PSO[key]
        for jbp in range(JB):
            va = va_sb[jbp // VA_CHUNK]
            vo = (jbp % VA_CHUNK) * H_PER_CORE * VA_W + 3 * VA_W
            nc.tensor.matmul(
                O[0:128, ds(nch * 512, 512)],
                va[:, ds(vo, VA_W)],
                ET[(3, 1)][jbp][:, ds(nch * 512, 512)],
                start=(jbp == 0), stop=(jbp == JB - 1),
                skip_group_check=True,
            )

    def emit_proj_inline(unit, slot=0, eng="vector"):
        # a 512-wide projection unit computed in a fill slot: the Y tile is
        # the previous step's dead S-tile (same WAR gate as fills); slot
        # picks which 512-col half of the dead tile hosts the unit
        ic, moch = unit
        inline_done.add(unit)
        tgt = dead_sp[-3]
        for hd2 in range(2):
            nc.tensor.matmul(
                tgt[:, ds(slot * 512, 512)],
                at_sb[hd2][:, ds(ic * 128, 128)],
                wt_sb[hd2][:, ds(moch * 512, 512)],
                start=(hd2 == 0), stop=(hd2 == 1),
                skip_group_check=True,
            )
        ysb = youtp.tile([128, 512], BF16, tag="y2", name="ysb2")
        if eng == "vector":
            nc.vector.tensor_copy(ysb[:], tgt[:, ds(slot * 512, 512)])
        elif eng == "gpsimd":
            nc.gpsimd.tensor_copy(ysb[:], tgt[:, ds(slot * 512, 512)])
        else:
            nc.scalar.copy(ysb[:], tgt[:, ds(slot * 512, 512)])
        nc.sync.dma_start(io["y"][ds(ic * 128, 128), ds(moch * 512, 512)],
                          ysb[:])

    # proj half-units inlined during block 3's PV0-heavy steps: ic 0-2
    # touch at_sb[:, 0:1024] only (t0/t1 norms, done during blocks 1-2).
    # The matmuls run right after QK and the copy rides the idle gpsimd,
    # so the dead-tile read retires before the next step's QK reuses the
    # slot
    inline_q = []
    for t in range(len(BLOCKS)):
        for h2 in range(2):
            ET[(t, h2)] = [None] * JB
        for jb in range(JB):
            pe = QK_PE
            if t == 3 and jb == JB - 1:
                # custom final step: h0's exp goes through the DVE fast-exp
                # so PV(3,0) closes and norm(3,0)'s recips start in-step;
                # h1's exp queues behind the recips (its consumers are all
                # in the tail)
                sps = emit_qk(3, 15)
                emit_exp(3, 15, 0, sps)
                emit_pv(3, 0, PV0_SCHED[15])
                emit_pv(3, 0, [15])
                emit_norm_recip(3, 0)
                e = etp.tile([128, IS], BF16, tag="et", name="e")
                nc.scalar.activation(e[:], sps[1][:], AF.Exp, scale=SCALE)
                ET[(3, 1)][15] = e
                dead_sp.append(sps[1])
                # PV(3,1)'s first 512-col sweep starts here, in the psS
                # slot freed by step 14's h1 tile (psO is still held by
                # PV(3,0) until its norm reads); its last matmul lands
                # just after h1's exp retires
                emit_pv31_sweep(0)
                continue
            inline = (t >= 2 and jb in (9, 10, 11, 12, 13, 14)
                      and bool(inline_q))
            if inline:
                # the unit's matmuls follow QK directly and its DVE copy
                # precedes this step's exps, so the dead-tile read retires
                # before the next step's QK reuses the psS slot
                sps = emit_qk(t, jb)
                emit_proj_inline(inline_q.pop(0), slot=0, eng="vector")
                emit_exp(t, jb, 0, sps)
                emit_exp(t, jb, 1, sps)
                pe += PV_PE
            else:
                emit_qk_exp(t, jb)
            if t > 0 and jb in (1, 8):
                # norm muls queue on the DVE right after this step's exp
                # (h1 is on ACT here); the psO slot frees ~1.4us in
                emit_norm_mul(t - 1, 0 if jb == 1 else 1)
            if t == 0:
                # no previous-block PV: spread own head0 one j-block per step
                if 1 <= jb < JB - 1:
                    emit_pv(0, 0, [jb - 1])
                    pe += PV_PE
                elif jb == JB - 1:
                    emit_pv(0, 0, [14])
                    pe += PV_PE
            else:
                if jb == 0:
                    # previous block's deferred last j-block + recip phase
                    emit_pv(t - 1, 0, [15])
                    emit_norm_recip(t - 1, 0)
                    pe += PV_PE
                if jb in PV1_SCHED:
                    jbps = PV1_SCHED[jb]
                    emit_pv(t - 1, 1, jbps)
                    pe += PV_PE * len(jbps)
                    if jb == 6:
                        emit_norm_recip(t - 1, 1)
                if jb in PV0_SCHED:
                    jbps = PV0_SCHED[jb]
                    emit_pv(t, 0, jbps)
                    pe += PV_PE * len(jbps)
            inline = False
            # fill-light steps stall on the fill gate (their [-3] S-tile's
            # exp is still in flight when the PE runs ahead); extra fills
            # behind that gate would only delay the next step's QK
            budget = (PE_TARGET_B0 if t == 0 else PE_TARGET) - pe
            if pe < 1500:
                budget = min(budget, (1 if inline else 3) * FILL_MAC)
            emit_fill(budget, half=1 if inline else None)

    # ---- tail ----
    # the nch0 half of PV(3,1) already ran inside step 15; the tail is:
    # nch1 sweep -> per-half recip/broadcast/mul -> hi proj units, with
    # the remaining lo proj units bridging every dependency wait.  hi
    # units issue a y DMA per 512-half: the final drain is transfer-bound,
    # so earlier first bytes shorten it.
    tail_ics = [ic for ic in range(16)
                if (ic, 0) not in inline_done or (ic, 1) not in inline_done]
    lo_ics = [ic for ic in tail_ics if ic < 8]    # at cols 0-1023: ready now
    hi_ics = [ic for ic in tail_ics if ic >= 8]   # need norm(3,*)

    def tail_proj(ic, k):
        mochs = tuple(m for m in (0, 1) if (ic, m) not in inline_done)
        emit_proj(ic, eng=("scalar" if k % 2 else "vector"), mochs=mochs)

    rr31 = normp.tile([1, IS], F32, tag="rr", name="rr31")
    bct31 = normp.tile([128, IS], F32, tag="bc", name="bc31")
    O31 = PSO[(3, 1)]
    nc.vector.reciprocal_approx_fast(rr31[0:1, 0:512], O31[0:1, 0:512])
    nc.gpsimd.partition_broadcast(bct31[:, 0:512], rr31[0:1, 0:512])
    emit_norm_mul(3, 0)
    nc.vector.tensor_mul(at_sb[1][ds(64, 64), ds(1024, 512)],
                         O31[64:128, 0:512], bct31[64:128, 0:512])
    k = 0
    for ic in lo_ics[:2]:
        tail_proj(ic, k); k += 1
    emit_pv31_sweep(1)
    O31b = PSO[(3, 1, "b")]
    for ic in lo_ics[2:]:
        tail_proj(ic, k); k += 1
    # the second norm half leads the DVE queue ahead of the hi copies so
    # hi ic12-15's at_sb gate clears while ic8-11 are still on the PE
    nc.vector.reciprocal_approx_fast(rr31[0:1, 512:IS], O31b[0:1, 512:IS])
    nc.gpsimd.partition_broadcast(bct31[:, 512:IS], rr31[0:1, 512:IS])
    nc.vector.tensor_mul(at_sb[1][ds(64, 64), ds(1024 + 512, 512)],
                         O31b[64:128, 512:IS], bct31[64:128, 512:IS])
    del ET[(3, 1)]
    for ic in hi_ics:
        tail_proj(ic, k); k += 1

_CACHED_NC = None


def _build():
    global _CACHED_NC
    if _CACHED_NC is not None:
        return _CACHED_NC
    nc = bacc.Bacc("TRN2", target_bir_lowering=False, debug=False,
                   num_devices=N_CORES)
    va_cols = JB // VA_CHUNK * H_PER_CORE * VA_W
    io = {
        "qt": nc.dram_tensor("qt", [2, 128, S], BF16,
                             kind="ExternalInput").ap(),
        "kt": nc.dram_tensor("kt", [2, 128, S], BF16,
                             kind="ExternalInput").ap(),
        "va": nc.dram_tensor("va", [VA_CHUNK, 128, va_cols], BF16,
                             kind="ExternalInput").ap(),
        "wt": nc.dram_tensor("wt", [2, 128, D_MODEL], BF16,
                             kind="ExternalInput").ap(),
        "y": nc.dram_tensor("y", [S, D_MODEL], BF16,
                            kind="ExternalOutput").ap(),
        "warm": nc.dram_tensor("warm", [1, 128], F32,
                               kind="ExternalOutput").ap(),
        "wexp": nc.dram_tensor("wexp", [1, 128], F32,
                               kind="ExternalOutput").ap(),
    }
    with tile.TileContext(nc) as tc:
        with ExitStack() as ctx:
            _trace(ctx, tc, io)
    nc.compile()
    _CACHED_NC = nc
    return nc


def _core_inputs(q, k, v, W, b, core):
    bb, g = divmod(core, 4)
    hd0 = g * H_PER_CORE * D_K  # 256 per group
    ncol = H_PER_CORE * D_K
    bf = ml_dtypes.bfloat16

    qt = np.ascontiguousarray(q[bb, :, hd0:hd0 + ncol].T).reshape(2, 128, S)
    kt = np.ascontiguousarray(k[bb, :, hd0:hd0 + ncol].T).reshape(2, 128, S)
    v_sl = v[bb, :, hd0:hd0 + ncol].reshape(S, H_PER_CORE, D_K)
    va = np.concatenate(
        [np.ones((S, H_PER_CORE, 1), np.float32),
         np.zeros((S, H_PER_CORE, 63), np.float32), v_sl], axis=2
    ).reshape(JB, 128, H_PER_CORE * VA_W).transpose(1, 0, 2).reshape(
        128, JB * H_PER_CORE * VA_W)
    va = va.reshape(128, VA_CHUNK, JB // VA_CHUNK * H_PER_CORE * VA_W)
    va = np.ascontiguousarray(va.transpose(1, 0, 2))
    wt = np.ascontiguousarray(W[:, hd0:hd0 + ncol].T).reshape(2, 128, D_MODEL)
    return {
        "qt": qt.astype(bf),
        "kt": kt.astype(bf),
        "va": va.astype(bf),
        "wt": wt.astype(bf),
    }


def run(inputs, trace=False, trace_kwargs=None):
    from concourse.bass_utils import run_bass_kernel_spmd

    q = np.asarray(inputs["q"], np.float32)
    k = np.asarray(inputs["k"], np.float32)
    v = np.asarray(inputs["v"], np.float32)
    W = np.asarray(inputs["W"], np.float32)
    b = np.asarray(inputs["b"], np.float32)

    nc = _build()
    in_maps = [_core_inputs(q, k, v, W, b, c) for c in range(N_CORES)]
    res = run_bass_kernel_spmd(nc, in_maps, core_ids=list(range(N_CORES)),
                               trace=trace, **(trace_kwargs or {}))
    out = np.empty((B, S, D_MODEL), np.float32)
    for bb in range(B):
        acc = res.results[bb * 4 + 0]["y"].astype(np.float32)
        for g in range(1, 4):
            acc = acc + res.results[bb * 4 + g]["y"].astype(np.float32)
        out[bb] = acc + b[None, :]
    return out, res


def kernel(**inputs):
    out, _ = run(inputs)
    return out
